# revision 9
# baseline (speedup 1.0000x reference)
"""Trainium2 Bass kernel v2 for nn_Evaluation_78383153152424.

Sharding: 8 cores = 2 batches x 4 D-groups (8 planes each). Zero collectives.

vs v1:
  - Stage-1 as two concurrent K=64 row-tiles ((0,0)/(64,0)), stage-2 as two
    concurrent M=64 col-tiles ((0,0)/(0,64)): ~2x PE throughput on the conv.
  - Gather muls are single-term FLAT DVE ops over [128, gn*XPAD] (measured:
    flat TT runs ~2.5x faster than 4-dim-AP TT). Pad columns carry reflected
    sim values and zero weights, so junk never escapes.
  - Center term folded: narrow(s=4) and wide(s=4) share the zero shift, so
    17 mul terms instead of 18.
  - Gather reduction engine per group is tunable: DVE flat add tree or PE
    identity-accumulate matmuls (kills the DVE-only tail).
  - relu1 on ACT, relu2 on DVE by default (per-site knobs).
  - 2 groups of 4 planes; per-block ps3 (1 bank) sim evac on ACT.
"""

import os
import sys
import functools

import numpy as np

for _p in ("/opt/trn_rl_repo", "/root/.axon_site/_ro/trn_rl_repo"):
    if os.path.isdir(_p) and _p not in sys.path:
        sys.path.append(_p)

import concourse.bass as bass
import concourse.tile as tile
from concourse import bacc, mybir
from concourse.bass_utils import run_bass_kernel_spmd

F16, F32 = mybir.dt.float16, mybir.dt.float32
AF = mybir.ActivationFunctionType
OP = mybir.AluOpType

B, G, D, H, W = 2, 8, 32, 128, 160
DG = 8
NCHUNK, RPC = 16, 8
CHUNK_F = RPC * W            # 1280
# row-aligned blocks (3/3/2 rows of W) so sim evac can write the halo-extended
# gb buffer with a plain contiguous copy
BLOCKS = [(0, 480), (480, 480), (960, 320)]
XPAD = W + 8                 # 168
GBROWS = 16                  # 4 halo + 8 own + 4 halo rows per chunk
GBF = GBROWS * W             # 2560 elements per gb partition


def _env(name, default):
    return os.environ.get(name, default)

GROUPS = [int(c) for c in _env("K_GROUPS", "44")]
GMAX = max(GROUPS)
WARMUP_MM = int(_env("K_WARMUP", "30"))
R1_ENG = _env("K_R1", "a" * 24)      # relu1 engine per (plane*3+k): a/v
R2_ENG = _env("K_R2", "v" * 24)      # relu2 engine per (plane*3+k): a/v
SE_ENG = _env("K_SE", "a" * 8)       # sim evac engine per (grp*3+k): a/v
TREE_ENG = _env("K_TREE", "vp")      # per group: v=DVE tree, p=PE matmuls
MUL_GP = int(_env("K_MULGP", "0"))   # how many mul terms per group on gpsimd
HALO_ENG = _env("K_HALO", "d")       # halo fill: c=shifted compute copy, d=DMA

# 17 gather terms: (shift_slot, dx, wall_slice). Wide terms s=0..8 use wall
# slice s (slice 4 pre-combined with narrow slice 13); narrow terms s!=4 use
# wall slice 9+s.
TERMS = []
for _s in range(9):
    _iy, _ix = _s // 3, _s % 3
    TERMS.append((2 * _iy, (_ix - 1) * 4))        # wide: dy=4(iy-1), slot=2iy
for _s in range(9):
    if _s == 4:
        continue
    _iy, _ix = _s // 3, _s % 3
    TERMS.append((_iy + 1, (_ix - 1) * 2))        # narrow: dy=2(iy-1), slot=iy+1
NTERM = len(TERMS)  # 17
YOFF = 4


def _ap(t, extra_off, dims):
    return bass.AP(tensor=t.tensor, offset=t.offset + extra_off, ap=dims)


@functools.lru_cache(maxsize=4)
def _build(zb0, zb1, zbsim):
    assert zb0 and zb1 and zbsim, "non-zero biases not supported in v2"
    nc = bacc.Bacc("TRN2", target_bir_lowering=False, debug=False, num_devices=8)

    x_ap = nc.dram_tensor("x", [NCHUNK, G, DG * CHUNK_F], F16,
                          kind="ExternalInput").ap()
    # wts cols: [s01: 128][ta: 64][l3e: 32][l3o: 32][ident: 128]
    wts_ap = nc.dram_tensor("wts", [128, 384], F16, kind="ExternalInput").ap()
    ow_ap = nc.dram_tensor("ow", [128, 19 * W], F16, kind="ExternalInput").ap()
    out_ap = nc.dram_tensor("out", [DG, H, W], F32, kind="ExternalOutput").ap()

    import contextlib
    with tile.TileContext(nc) as tc, contextlib.ExitStack() as ctx:
        wp = ctx.enter_context(tc.tile_pool(name="wp", bufs=1))
        xp = ctx.enter_context(tc.tile_pool(name="xp", bufs=2))
        hp = ctx.enter_context(tc.tile_pool(name="hp", bufs=3))
        h2p = ctx.enter_context(tc.tile_pool(name="h2p", bufs=4))
        sfp = ctx.enter_context(tc.tile_pool(name="sfp", bufs=2))
        yvp = ctx.enter_context(tc.tile_pool(name="yvp", bufs=2))
        ptp = ctx.enter_context(tc.tile_pool(name="ptp", bufs=2))
        ofp = ctx.enter_context(tc.tile_pool(name="ofp", bufs=2))
        ps1p = ctx.enter_context(tc.tile_pool(name="ps1p", bufs=2, space="PSUM"))
        ps2p = ctx.enter_context(tc.tile_pool(name="ps2p", bufs=1, space="PSUM"))
        ps3p = ctx.enter_context(tc.tile_pool(name="ps3p", bufs=2, space="PSUM"))

        wts = wp.tile([128, 384], F16)
        nc.sync.dma_start(out=wts[:], in_=wts_ap[:])
        s01 = wts[:, 0:128]
        ta = wts[:, 128:192]
        l3e = wts[:, 192:224]
        l3o = wts[:, 224:256]
        ident = wts[:, 256:384]
        ow = wp.tile([128, 19 * W], F16)
        nc.sync.dma_start(out=ow[:], in_=ow_ap[:])
        offs, wgts = ow[:, 0:18 * W], ow[:, 18 * W:19 * W]

        # x loads in 2-plane chunks so conv starts early
        xts = {}
        plane0 = 0
        for grp, gn in enumerate(GROUPS):
            xt = xp.tile([128, gn * CHUNK_F], F16, tag="x", name=f"xt{grp}")
            for jj in range(0, gn, 2):
                nn = min(2, gn - jj)
                nc.sync.dma_start(
                    out=xt[:, jj * CHUNK_F:(jj + nn) * CHUNK_F],
                    in_=x_ap[:, :, (plane0 + jj) * CHUNK_F:
                             (plane0 + jj + nn) * CHUNK_F])
            xts[grp] = xt
            plane0 += gn

        # HAM warmup while x loads
        ps_w = ps1p.tile([128, 1024], F32, tag="ps1", name="ps_warm")
        for _ in range(WARMUP_MM):
            nc.tensor.matmul(ps_w[:, 0:128], ident, wts[:, 0:128],
                             start=True, stop=True)
        del ps_w

        # ---- gather weights ----
        # wall[s, x] = offs[s, x] * 0.5 * weight[x]; then wall[4] += wall[13]
        wgth = wp.tile([128, W], F16)
        nc.vector.tensor_scalar_mul(wgth[:], wgts, 0.5)
        wall = wp.tile([128, 18 * W], F16)
        wgth_b = _ap(wgth, 0, [list(wgth.ap[0]), [0, 18], [1, W]])
        nc.vector.tensor_tensor(
            wall[:, :].rearrange("p (s x) -> p s x", s=18),
            offs.rearrange("p (s x) -> p s x", s=18),
            wgth_b, OP.mult)
        nc.vector.tensor_tensor(wall[:, 4 * W:5 * W], wall[:, 4 * W:5 * W],
                                wall[:, 13 * W:14 * W], OP.add)
        # wallp: XPAD layout with zero pad cols
        wallp = wp.tile([128, NTERM * XPAD], F16)
        nc.gpsimd.memset(_ap(wallp, 0,
                         [list(wallp.ap[0]), [XPAD, NTERM], [1, 4]]), 0)
        nc.gpsimd.memset(_ap(wallp, 164,
                         [list(wallp.ap[0]), [XPAD, NTERM], [1, 4]]), 0)
        # copy the 17 used slices into wallp (dst offset 4 within each XPAD run)
        # do it in 2 contiguous runs: slices 0..12 (wall cols 0..13W) and 14..17
        nc.vector.tensor_copy(
            _ap(wallp, 4, [list(wallp.ap[0]), [XPAD, 13], [1, W]]),
            _ap(wall, 0, [list(wall.ap[0]), [W, 13], [1, W]]))
        nc.vector.tensor_copy(
            _ap(wallp, 13 * XPAD + 4, [list(wallp.ap[0]), [XPAD, 4], [1, W]]),
            _ap(wall, 14 * W, [list(wall.ap[0]), [W, 4], [1, W]]))
        # wrep: replicate wallp over planes -> [t][j][XPAD]
        wrep = wp.tile([128, NTERM * GMAX * XPAD], F16)
        for dd in range(GMAX):
            nc.gpsimd.dma_start(
                out=_ap(wrep, dd * XPAD,
                        [list(wrep.ap[0]), [GMAX * XPAD, NTERM], [1, XPAD]]),
                in_=_ap(wallp, 0,
                        [list(wallp.ap[0]), [XPAD, NTERM], [1, XPAD]]))

        plane0 = 0
        for grp, gn in enumerate(GROUPS):
            planes = list(range(plane0, plane0 + gn))
            plane0 += gn
            xt = xts[grp]
            gX = gn * XPAD
            # gb: halo-extended per-chunk sim rows [4 halo | 8 own | 4 halo]*W
            gb = sfp.tile([16 * gn, GBF], F16, tag="gb")

            # ---- conv chain: 2-plane rounds share each stationary ----
            for k, (fo, fn) in enumerate(BLOCKS):
                ps3 = ps3p.tile([32 * ((gn + 1) // 2), 512], F32, tag="ps3")
                for jp in range(0, gn, 2):
                    js = [jp] if jp + 1 >= gn else [jp, jp + 1]
                    ps1s, h1s = [], []
                    for j in js:
                        xv = xt[:, j * CHUNK_F:(j + 1) * CHUNK_F]
                        ps1 = ps1p.tile([128, 1024], F32, tag="ps1")
                        nc.tensor.matmul(ps1[:, 0:fn], s01[0:64, :],
                                         xv[0:64, fo:fo + fn],
                                         start=True, stop=True,
                                         tile_position=(0, 0))
                        nc.tensor.matmul(ps1[:, 512:512 + fn], s01[64:128, :],
                                         xv[64:128, fo:fo + fn],
                                         start=True, stop=True,
                                         tile_position=(64, 0))
                        ps1s.append(ps1)
                    for j, ps1 in zip(js, ps1s):
                        h1 = hp.tile([128, 1024], F16, tag="h1")
                        r1e = R1_ENG[((planes[j]) * 3 + k) % len(R1_ENG)]
                        if fn == 512:
                            if r1e == "a":
                                nc.scalar.activation(h1[:, 0:1024],
                                                     ps1[:, 0:1024], AF.Relu)
                            else:
                                nc.vector.tensor_scalar_max(
                                    h1[:, 0:1024], ps1[:, 0:1024], 0.0)
                        else:
                            for so in (0, 512):
                                if r1e == "a":
                                    nc.scalar.activation(
                                        h1[:, so:so + fn],
                                        ps1[:, so:so + fn], AF.Relu)
                                else:
                                    nc.vector.tensor_scalar_max(
                                        h1[:, so:so + fn],
                                        ps1[:, so:so + fn], 0.0)
                        h1s.append(h1)
                    ps2 = ps2p.tile([128, 1024], F32, tag="ps2")
                    for i, (j, h1) in enumerate(zip(js, h1s)):
                        co = 512 * i
                        nc.tensor.matmul(ps2[0:64, co:co + fn], ta,
                                         h1[:, 0:fn], start=True, stop=True,
                                         tile_position=(0, 0))
                        nc.tensor.matmul(ps2[64:128, co:co + fn], ta,
                                         h1[:, 512:512 + fn],
                                         start=True, stop=True,
                                         tile_position=(0, 64))
                    h2 = h2p.tile([128, 1024], F16, tag="h2")
                    r2e = R2_ENG[((planes[jp]) * 3 + k) % len(R2_ENG)]
                    if fn == 512 and len(js) == 2:
                        if r2e == "a":
                            nc.scalar.activation(h2[:, 0:1024], ps2[:, 0:1024],
                                                 AF.Relu)
                        else:
                            nc.vector.tensor_scalar_max(h2[:, 0:1024],
                                                        ps2[:, 0:1024], 0.0)
                    else:
                        for i in range(len(js)):
                            co = 512 * i
                            if r2e == "a":
                                nc.scalar.activation(h2[:, co:co + fn],
                                                     ps2[:, co:co + fn], AF.Relu)
                            else:
                                nc.vector.tensor_scalar_max(
                                    h2[:, co:co + fn], ps2[:, co:co + fn], 0.0)
                    pr = jp // 2
                    for i, j in enumerate(js):
                        nc.tensor.matmul(ps3[32 * pr:32 * pr + 32, 0:fn],
                                         l3e if i == 0 else l3o,
                                         h2[:, 512 * i:512 * i + fn],
                                         start=(i == 0), stop=(i == len(js) - 1),
                                         tile_position=(0, 32 * pr))
                if SE_ENG[(grp * 3 + k) % len(SE_ENG)] == "a":
                    nc.scalar.copy(gb[:, 4 * W + fo:4 * W + fo + fn],
                                   ps3[0:16 * gn, 0:fn])
                else:
                    nc.vector.tensor_copy(gb[:, 4 * W + fo:4 * W + fo + fn],
                                          ps3[0:16 * gn, 0:fn])

            # ---- fill gb halos, then SBUF->SBUF 5-shift reads into yvall ----
            # interior halos: top rows 0-3 <- prev partition own rows 4-7,
            # bottom rows 12-15 <- next partition own rows 0-3
            if HALO_ENG == "c":
                nc.vector.tensor_copy(gb[1:16 * gn, 0:4 * W],
                                      gb[0:16 * gn - 1, 8 * W:12 * W])
                nc.vector.tensor_copy(gb[0:16 * gn - 1, 12 * W:16 * W],
                                      gb[1:16 * gn, 4 * W:8 * W])
            else:
                for j in range(gn):
                    nc.gpsimd.dma_start(
                        out=gb[16 * j + 1:16 * j + 16, 0:4 * W],
                        in_=gb[16 * j:16 * j + 15, 8 * W:12 * W])
                    nc.gpsimd.dma_start(
                        out=gb[16 * j:16 * j + 15, 12 * W:16 * W],
                        in_=gb[16 * j + 1:16 * j + 16, 4 * W:8 * W])
            # chunk-0 top reflect (rows -4..-1 = own rows 4,3,2,1) and
            # chunk-15 bottom reflect (rows 128..131 = own rows 6,5,4,3)
            GP = int(gb.ap[0][0])        # partition pitch (flat encoding)
            nc.gpsimd.dma_start(
                out=_ap(gb, 0, [[16 * GP, gn], [W, 4], [1, W]]),
                in_=_ap(gb, 8 * W, [[16 * GP, gn], [-W, 4], [1, W]]))
            nc.gpsimd.dma_start(
                out=_ap(gb, 15 * GP + 12 * W, [[16 * GP, gn], [W, 4], [1, W]]),
                in_=_ap(gb, 15 * GP + 10 * W, [[16 * GP, gn], [-W, 4], [1, W]]))

            yvall = yvp.tile([128, YOFF + 5 * gX + 8], F16, tag="yvall")
            nc.gpsimd.memset(yvall[:, 0:YOFF], 0)
            nc.gpsimd.memset(yvall[:, YOFF + 5 * gX:YOFF + 5 * gX + 8], 0)
            # per (plane, shift): DMA reading (c, rr, x) from gb -> (r, x)
            for j in range(gn):
                src_base = gb[16 * j:16 * j + 16, 0:1]
                for s in range(5):
                    srcp = bass.AP(tensor=src_base.tensor,
                                   offset=src_base.offset + 2 * s * W,
                                   ap=[[src_base.ap[0][0], 16], [W, 8], [1, W]])
                    dst = _ap(yvall, YOFF + j * XPAD + s * gX + 4,
                              [list(yvall.ap[0]), [1, W]])
                    eng = (nc.sync, nc.gpsimd, nc.sync, nc.gpsimd,
                           nc.sync)[s]
                    eng.dma_start(out=dst, in_=srcp)
            # x-edge reflect pads for all shifts/planes
            lp_d = _ap(yvall, YOFF, [list(yvall.ap[0]), [XPAD, 5 * gn], [1, 4]])
            lp_s = _ap(yvall, YOFF + 8,
                       [list(yvall.ap[0]), [XPAD, 5 * gn], [-1, 4]])
            nc.vector.tensor_copy(lp_d, lp_s)
            rp_d = _ap(yvall, YOFF + 164,
                       [list(yvall.ap[0]), [XPAD, 5 * gn], [1, 4]])
            rp_s = _ap(yvall, YOFF + 162,
                       [list(yvall.ap[0]), [XPAD, 5 * gn], [-1, 4]])
            nc.vector.tensor_copy(rp_d, rp_s)

            # ---- gather muls: P[t] = wrep[t] * yvall[slot_t, shifted dx] ----
            P = ptp.tile([128, NTERM * gX], F16, tag="gtmp")
            for i, (slot, dx) in enumerate(TERMS):
                srcv = _ap(yvall, YOFF + slot * gX + dx,
                           [list(yvall.ap[0]), [1, gX]])
                w_b = _ap(wrep, i * GMAX * XPAD,
                          [list(wrep.ap[0]), [1, gX]])
                dst = P[:, i * gX:(i + 1) * gX]
                eng = nc.gpsimd if i < MUL_GP else nc.vector
                eng.tensor_tensor(dst, w_b, srcv, OP.mult)

            if TREE_ENG[grp % len(TREE_ENG)] == "p":
                # PE identity-accumulate: 17 terms x 2 halves into 2 psum banks
                psg = ps1p.tile([128, 1024], F32, tag="ps1", name=f"psg{grp}")
                hw = gX // 2
                for t in range(NTERM):
                    nc.tensor.matmul(psg[:, 0:hw], ident,
                                     P[:, t * gX:t * gX + hw],
                                     start=(t == 0), stop=(t == NTERM - 1))
                for t in range(NTERM):
                    nc.tensor.matmul(psg[:, 512:512 + hw], ident,
                                     P[:, t * gX + hw:(t + 1) * gX],
                                     start=(t == 0), stop=(t == NTERM - 1))
                of32 = ofp.tile([128, gX], F32, tag="of32")
                nc.scalar.copy(of32[:, 0:hw], psg[:, 0:hw])
                nc.scalar.copy(of32[:, hw:gX], psg[:, 512:512 + hw])
            else:
                # DVE flat add tree over 17 slices: 16 -> 8 -> 4 -> 2 -> 1, +last
                of32 = ofp.tile([128, gX], F32, tag="of32")
                nc.vector.tensor_tensor(P[:, 0:8 * gX], P[:, 0:8 * gX],
                                        P[:, 8 * gX:16 * gX], OP.add)
                nc.vector.tensor_tensor(P[:, 0:4 * gX], P[:, 0:4 * gX],
                                        P[:, 4 * gX:8 * gX], OP.add)
                nc.vector.tensor_tensor(P[:, 0:2 * gX], P[:, 0:2 * gX],
                                        P[:, 2 * gX:4 * gX], OP.add)
                nc.vector.tensor_tensor(P[:, 0:gX], P[:, 0:gX],
                                        P[:, gX:2 * gX], OP.add)
                nc.vector.tensor_tensor(of32[:, :], P[:, 0:gX],
                                        P[:, 16 * gX:17 * gX], OP.add)
            nc.scalar.dma_start(
                out=out_ap[planes[0]:planes[0] + gn].rearrange("d h x -> h d x"),
                in_=_ap(of32, 4, [list(of32.ap[0]), [XPAD, gn], [1, W]]))

    nc.compile()
    return nc


def _pack_weights(w0, bn0_scale, bn0_bias, w1, bn1_scale, bn1_bias, w_sim, b_sim):
    w0f = (w0 * bn0_scale[:, None]).astype(np.float32)
    w1f = (w1 * bn1_scale[:, None]).astype(np.float32)
    s01 = np.zeros((128, 128), np.float16)
    for half in range(2):
        for a in range(8):
            for g in range(G):
                for o in range(16):
                    s01[64 * half + 8 * a + g, 16 * a + o] = w0f[o, g]
    ta = np.zeros((128, 64), np.float16)
    for a in range(8):
        for o in range(16):
            for q in range(8):
                ta[16 * a + o, 8 * a + q] = w1f[q, o]
    l3 = np.zeros((128, 64), np.float16)
    for c in range(NCHUNK):
        l3[c * 8:c * 8 + 8, c] = w_sim[0, :]          # l3even: cols 0-15
        l3[c * 8:c * 8 + 8, 32 + 16 + c] = w_sim[0, :]  # l3odd: cols 48-63
    ident = np.eye(128, dtype=np.float16)
    return np.hstack([s01, ta, l3, ident])


def prepare(x1, offset, weight, w0, bn0_scale, bn0_bias, w1, bn1_scale, bn1_bias,
            w_sim, b_sim):
    x1 = np.asarray(x1); offset = np.asarray(offset); weight = np.asarray(weight)
    w0 = np.asarray(w0); bn0_scale = np.asarray(bn0_scale)
    bn0_bias = np.asarray(bn0_bias); w1 = np.asarray(w1)
    bn1_scale = np.asarray(bn1_scale); bn1_bias = np.asarray(bn1_bias)
    w_sim = np.asarray(w_sim); b_sim = np.asarray(b_sim)

    wts = _pack_weights(w0, bn0_scale, bn0_bias, w1, bn1_scale, bn1_bias,
                        w_sim, b_sim)
    zb0 = bool(np.all(bn0_bias == 0))
    zb1 = bool(np.all(bn1_bias == 0))
    zbsim = bool(np.all(b_sim == 0))
    nc = _build(zb0, zb1, zbsim)

    in_maps = []
    for core in range(8):
        b, kd = divmod(core, 4)
        ow = np.concatenate([offset[b].transpose(1, 0, 2).reshape(H, 18 * W),
                             weight[b, 0]], axis=1).astype(np.float16)
        xs = x1[b, :, kd * DG:(kd + 1) * DG].astype(np.float16)
        xs = xs.reshape(G, DG, NCHUNK, RPC, W)
        xs = np.ascontiguousarray(xs.transpose(2, 0, 1, 3, 4)).reshape(
            NCHUNK, G, DG * CHUNK_F)
        in_maps.append({"x": xs, "wts": wts, "ow": ow})
    return nc, in_maps


def kernel(x1, offset, weight, w0, bn0_scale, bn0_bias, w1, bn1_scale, bn1_bias,
           w_sim, b_sim):
    nc, in_maps = prepare(x1, offset, weight, w0, bn0_scale, bn0_bias, w1,
                          bn1_scale, bn1_bias, w_sim, b_sim)
    res = run_bass_kernel_spmd(nc, in_maps, list(range(8)))
    out = np.empty((B, D, H, W), np.float32)
    for core in range(8):
        b, kd = divmod(core, 4)
        out[b, kd * DG:(kd + 1) * DG] = res.results[core]["out"]
    return out

